# revision 18
# baseline (speedup 1.0000x reference)
"""Trainium2 Bass kernel for the CLAPP layer, SPMD over 8 NeuronCores.

Math (reference, fp32):
    cur       = W_fc @ inp                       [H]
    new_mem   = 0.9*mem + cur
    spk       = (new_mem > 1).f32
    mem_out   = new_mem - spk
    retro     = spk @ W_pred                     [H]
    dW        = bf*(outer(feedback*sur(mem_out), inp)
                    + outer(retro*sur(prev_mem), prev_inp))   [H, I]
    dW_pred   = bf*outer(spk, prev_spk)          [H, H]
    loss      = -bf*dot(spk, feedback)/H
    fb_new    = W_pred @ spk                     [H]
    where sur(x) = 1/(pi*(1+(pi*x)^2))

Sharding: W_fc and W_pred row-sharded over 8 cores (S = H/8 rows each).
Core s computes cur_s/spk_s/mem_out_s, its rows of dW/dW_pred, a retro
partial (reduce-scattered so each core receives its own retro slice, with
the loss partial folded into an extra column), and fb_new_s.
Collectives: AllGather(spk_s) and ReduceScatter(retro partials + loss).

Engine split (fp32 matmul costs 4 PE-cycles/row, so the big outer
products are NOT done on the tensor engine):
    PE : cur (W_fc^T stationary), retro partial (spk stationary)
    ACT: a-part of dW and all of dW_pred via activation(Copy, scale=[P,1])
    DVE: dynamics, fb_new row-dots (scalar_tensor_tensor + accum_out),
         b-part of dW (scalar_tensor_tensor, in-place add)
    POOL: collectives, partition_broadcast replication, output DMAs

Per-core vector layout is column-major ("cm"): a length-S vector lives in
an SBUF tile [128, S/128] with v[m*128+p] at [p, m]; partition p of block
m is exactly row m*128+p of the core's W rows, so cm columns are directly
usable as per-partition scale operands.

Per-core inputs from the host:
    wfcT   [I, S]   = W_fc[s*S:(s+1)*S, :].T    (PE contracts over rows)
    wpred  [S, H]   = W_pred[s*S:(s+1)*S, :]
    inp_cm [128, I/128]                          (rhs columns for cur)
    inp_pi [2, I]   = stack(inp, prev_inp)       (rows, replicated on-chip)
    mem_cm/pmem_cm/fbk_cm [128, S/128] per-core slices in cm layout
    pspk   [1, H]                                (row, replicated on-chip)
Outputs per core: spk/mem_out/fb_new in cm [128, S/128] (host transposes
back), loss [1,1], dW [S, I], dW_pred [S, H].
"""

import sys

if "/opt/trn_rl_repo" not in sys.path:
    sys.path.insert(0, "/opt/trn_rl_repo")

import numpy as np

import concourse.bass as bass
import concourse.bass_isa as bass_isa
import concourse.mybir as mybir
import concourse.tile as tile
from concourse import bacc
from concourse.bass_utils import run_bass_kernel_spmd

F32 = mybir.dt.float32
BF16 = mybir.dt.bfloat16
ALU = mybir.AluOpType
ACTF = mybir.ActivationFunctionType

NCORES = 8
P = 128
NB = 512  # psum bank width in f32


def build(H=8192, I=8192, bf=1.0, beta=0.9):
    """Build and compile the SPMD Bass module (identical program on all cores)."""
    S = H // NCORES          # rows of W_fc/W_pred per core
    assert S % P == 0 and I % P == 0 and H % NCORES == 0
    KC = I // P              # contraction chunks for cur
    MC = S // P              # 128-row blocks per core
    CC = 2048 if H % 2048 == 0 else min(512, S)  # W_pred streaming col chunk
    NQ = H // CC
    NBr = min(NB, CC)        # retro psum chunk
    assert CC % NBr == 0 and S % NBr == 0
    SW = 2048 if I % 2048 == 0 else I   # dW/dW_pred staging width
    NH = I // SW
    pi = float(np.pi)
    pi15 = float(np.pi ** 1.5)
    bf_f = float(bf)

    nc = bacc.Bacc("TRN2", target_bir_lowering=False, debug=False,
                   num_devices=NCORES)

    # ---------------- DRAM I/O ----------------
    wfcT_d = nc.dram_tensor("wfcT", [I, S], F32, kind="ExternalInput")
    wpred_d = nc.dram_tensor("wpred", [S, H], F32, kind="ExternalInput")
    inpcm_d = nc.dram_tensor("inp_cm", [P, KC], F32, kind="ExternalInput")
    inppi_d = nc.dram_tensor("inp_pi", [2, I], F32, kind="ExternalInput")
    mem_d = nc.dram_tensor("mem_cm", [P, MC], F32, kind="ExternalInput")
    pmem_d = nc.dram_tensor("pmem_cm", [P, MC], F32, kind="ExternalInput")
    fbk_d = nc.dram_tensor("fbk_cm", [P, MC], F32, kind="ExternalInput")
    pspk_d = nc.dram_tensor("pspk", [1, H], F32, kind="ExternalInput")

    spk_o = nc.dram_tensor("spk_o", [P, MC], F32, kind="ExternalOutput")
    memout_o = nc.dram_tensor("memout_o", [P, MC], F32, kind="ExternalOutput")
    loss_o = nc.dram_tensor("loss_o", [1, 1], F32, kind="ExternalOutput")
    dw_o = nc.dram_tensor("dw_o", [S, I], F32, kind="ExternalOutput")
    dwp_o = nc.dram_tensor("dwp_o", [S, H], F32, kind="ExternalOutput")
    fbn_o = nc.dram_tensor("fbn_o", [P, MC], F32, kind="ExternalOutput")

    RG = [list(range(NCORES))]

    with tile.TileContext(nc) as tc:
        with (
            tc.tile_pool(name="wfc", bufs=3) as wfcp,
            tc.tile_pool(name="wpr", bufs=6) as wprp,
            tc.tile_pool(name="stage", bufs=5) as stagep,
            tc.tile_pool(name="rstage", bufs=2) as rstagep,
            tc.tile_pool(name="reps", bufs=2) as repsp,
            tc.tile_pool(name="rows", bufs=1) as rowsp,
            tc.tile_pool(name="small", bufs=1) as small,
            tc.tile_pool(name="fba", bufs=2) as fbap,
            tc.tile_pool(name="psum", bufs=1, space="PSUM") as psump,
            tc.tile_pool(name="dram", bufs=1, space="DRAM") as dramp,
        ):
            # persistent tiles; cm vectors are [P, MC] (tiny SBUF depth)
            inpcm_sb = small.tile([P, KC], F32, tag="inpcm")
            mem_sb = small.tile([P, MC], F32, tag="mem")
            pmem_sb = small.tile([P, MC], F32, tag="pmem")
            fbk_sb = small.tile([P, MC], F32, tag="fbk")
            cur_sb = small.tile([P, MC], F32, tag="cur")
            t0_sb = small.tile([P, MC], F32, tag="t0")
            nm_sb = small.tile([P, MC], F32, tag="nm")
            spk_sb = small.tile([P, MC], F32, tag="spk")
            memout_sb = small.tile([P, MC], F32, tag="memout")
            sm1_sb = small.tile([P, MC], F32, tag="sm1")
            sm2_sb = small.tile([P, MC], F32, tag="sm2")
            smr_sb = small.tile([P, MC], F32, tag="smr")
            sp1_sb = small.tile([P, MC], F32, tag="sp1")
            sp2_sb = small.tile([P, MC], F32, tag="sp2")
            spr_sb = small.tile([P, MC], F32, tag="spr")
            atmp_sb = small.tile([P, MC], F32, tag="atmp")
            acm_sb = small.tile([P, MC], F32, tag="acm")
            btmp_sb = small.tile([P, MC], F32, tag="btmp")
            bcm_sb = small.tile([P, MC], F32, tag="bcm")
            spkbf_cm = small.tile([P, MC], F32, tag="spkbfcm")
            rs_cm = small.tile([P, MC], F32, tag="rscm")
            lossp0_sb = small.tile([P, 1], F32, tag="lossp0")
            lossp_sb = small.tile([P, 1], F32, tag="lossp")
            lossar_sb = small.tile([P, 1], F32, tag="lossar")
            fbn_sb = small.tile([P, MC], F32, tag="fbn")

            # collective bounce buffers (internal DRAM)
            ag_in = dramp.tile([1, S], F32, tag="ag_in")
            ag_out = dramp.tile([NCORES, S], F32, tag="ag_out")
            rs_in = dramp.tile([NCORES, S + 1], F32, tag="rs_in")
            rs_out = dramp.tile([1, S + 1], F32, tag="rs_out")

            # ---- phase 0: small input loads
            nc.sync.dma_start(inpcm_sb[:], inpcm_d[:])
            nc.sync.dma_start(mem_sb[:], mem_d[:])
            nc.sync.dma_start(pmem_sb[:], pmem_d[:])
            nc.sync.dma_start(fbk_sb[:], fbk_d[:])

            # sur(prev_mem) early (off the critical path)
            nc.scalar.activation(sp1_sb[:], pmem_sb[:], ACTF.Square, scale=pi15)
            nc.vector.tensor_scalar(sp2_sb[:], sp1_sb[:], pi, None, op0=ALU.add)
            nc.vector.reciprocal(spr_sb[:], sp2_sb[:])

            # ---- phase 1: stream W_fc^T; cur (cm layout) in one psum bank
            cur_ps = psump.tile([P, MC], F32, tag="curps", bufs=1)
            for k in range(KC):
                wt = wfcp.tile([P, S], F32, tag="wfc")
                nc.sync.dma_start(wt[:], wfcT_d[k * P:(k + 1) * P, :])
                for m in range(MC):
                    # one accumulation group spans the whole bank (start
                    # zeroes the full 2KB zero region)
                    nc.tensor.matmul(
                        cur_ps[:, m:m + 1],
                        wt[:, m * P:(m + 1) * P],
                        inpcm_sb[:, k:k + 1],
                        start=(k == 0 and m == 0),
                        stop=(k == KC - 1 and m == MC - 1),
                    )

            # ---- phase 2: neuron dynamics (cm layout, full 128 lanes)
            nc.vector.tensor_copy(cur_sb[:], cur_ps[:])
            nc.vector.tensor_scalar(t0_sb[:], mem_sb[:], beta, None, op0=ALU.mult)
            nc.vector.tensor_tensor(nm_sb[:], t0_sb[:], cur_sb[:], op=ALU.add)
            nc.vector.tensor_scalar(spk_sb[:], nm_sb[:], 1.0, None, op0=ALU.is_gt)
            nc.vector.tensor_tensor(memout_sb[:], nm_sb[:], spk_sb[:], op=ALU.subtract)

            # ---- phase 3: spk allgather; replicate spk across partitions
            nc.gpsimd.dma_start(
                ag_in[0, :].rearrange("(m p) -> p m", p=P), spk_sb[:]
            )
            nc.gpsimd.collective_compute(
                "AllGather", ALU.bypass, replica_groups=RG,
                ins=[ag_in.opt()], outs=[ag_out.opt()],
            )
            spkfull_row = rowsp.tile([1, H], F32, tag="row", name="spkfull_row")
            nc.sync.dma_start(spkfull_row[:], ag_out[:].rearrange("r s -> (r s)"))
            spkrep = repsp.tile([P, H], F32, tag="rep", name="spkrep")
            nc.gpsimd.partition_broadcast(spkrep[:], spkfull_row[:])

            # a = bf * feedback_s * sur(mem_out); bf*spk  (cm layout)
            nc.scalar.activation(sm1_sb[:], memout_sb[:], ACTF.Square, scale=pi15)
            nc.vector.tensor_scalar(sm2_sb[:], sm1_sb[:], pi, None, op0=ALU.add)
            nc.vector.reciprocal(smr_sb[:], sm2_sb[:])
            nc.vector.tensor_tensor(atmp_sb[:], fbk_sb[:], smr_sb[:], op=ALU.mult)
            nc.vector.tensor_scalar(acm_sb[:], atmp_sb[:], bf_f, None, op0=ALU.mult)
            nc.vector.tensor_scalar(spkbf_cm[:], spk_sb[:], bf_f, None, op0=ALU.mult)

            # loss partial: sum(-bf/H * spk_s * feedback_s), all-reduced over
            # partitions, then folded into the reduce-scatter as column S
            nc.vector.tensor_tensor(t0_sb[:], spk_sb[:], fbk_sb[:], op=ALU.mult)
            nc.vector.tensor_reduce(
                lossp0_sb[:], t0_sb[:], axis=mybir.AxisListType.X, op=ALU.add
            )
            nc.vector.tensor_scalar(
                lossp_sb[:], lossp0_sb[:], -bf_f / float(H), None, op0=ALU.mult
            )
            nc.gpsimd.partition_all_reduce(
                lossar_sb[:], lossp_sb[:], channels=P,
                reduce_op=bass_isa.ReduceOp.add,
            )
            nc.gpsimd.dma_start(rs_in[:, S:S + 1], lossar_sb[0:NCORES, :])

            # small outputs that are already final
            nc.gpsimd.dma_start(spk_o[:], spk_sb[:])
            nc.gpsimd.dma_start(memout_o[:], memout_sb[:])

            # replicate prev_spk across partitions for the dW_pred outers
            pspk_row = rowsp.tile([1, H], F32, tag="row", name="pspk_row")
            nc.sync.dma_start(pspk_row[:], pspk_d[:])
            pspkrep = repsp.tile([P, H], F32, tag="rep", name="pspkrep")
            nc.gpsimd.partition_broadcast(pspkrep[:], pspk_row[:])

            # ---- phase 4: W_pred stream (retro on PE + fb_new on DVE).
            # The output streams are deliberately NOT started here so the
            # W_pred input stream gets the full DMA bandwidth and the
            # reduce-scatter fires as early as possible.
            fb_prev = [None] * MC
            for q in range(NQ):
                rps = [
                    psump.tile([1, NBr], F32, tag=f"retro{j}", bufs=1,
                               name=f"retrops{q}_{j}")
                    for j in range(CC // NBr)
                ]
                for r in range(MC):
                    wt = wprp.tile([P, CC], F32, tag="wpr", name=f"wpr{q}_{r}")
                    nc.sync.dma_start(
                        wt[:], wpred_d[r * P:(r + 1) * P, q * CC:(q + 1) * CC]
                    )
                    for j in range(CC // NBr):
                        nc.tensor.matmul(
                            rps[j][:],
                            spk_sb[:, r:r + 1],
                            wt[:, j * NBr:(j + 1) * NBr],
                            start=(r == 0),
                            stop=(r == MC - 1),
                        )
                    # fb_new partial: row-dot of this W_pred tile with spk,
                    # fused multiply + free-axis accumulate (in-place)
                    red = fbap.tile([P, 1], F32, tag=f"fbr{r}", name=f"fbr{q}_{r}")
                    nc.vector.scalar_tensor_tensor(
                        wt[:], wt[:], 1.0, spkrep[:, q * CC:(q + 1) * CC],
                        op0=ALU.mult, op1=ALU.mult, accum_out=red[:],
                    )
                    if q == 0:
                        fb_prev[r] = red
                    else:
                        facc = fbap.tile([P, 1], F32, tag=f"fba{r}",
                                         name=f"fba{q}_{r}")
                        nc.vector.tensor_tensor(
                            facc[:], fb_prev[r][:], red[:], op=ALU.add
                        )
                        fb_prev[r] = facc
                # retro psum -> sbuf staging -> rs_in slices
                for j in range(CC // NBr):
                    rst = rstagep.tile([1, NBr], F32, tag="rst", name=f"rst{q}_{j}")
                    nc.vector.tensor_copy(rst[:], rps[j][:])
                    n0 = q * CC + j * NBr
                    nc.gpsimd.dma_start(
                        rs_in[n0 // S: n0 // S + 1, n0 % S: n0 % S + NBr], rst[:]
                    )
            for r in range(MC):
                nc.vector.tensor_copy(fbn_sb[:, r:r + 1], fb_prev[r][:])

            # ---- phase 4c: reduce-scatter; b = bf * retro_s * sur(prev_mem)
            nc.gpsimd.collective_compute(
                "ReduceScatter", ALU.add, replica_groups=RG,
                ins=[rs_in.opt()], outs=[rs_out.opt()],
            )

            # dW_pred rows = bf*spk[p] * prev_spk (ACT), streamed out while
            # the reduce-scatter is in flight
            for m in range(MC):
                for h in range(NH):
                    st = stagep.tile([P, SW], F32, tag="stage",
                                     name=f"stp{m}_{h}")
                    nc.scalar.activation(
                        st[:], pspkrep[:, h * SW:(h + 1) * SW], ACTF.Copy,
                        scale=spkbf_cm[:, m:m + 1],
                    )
                    nc.gpsimd.dma_start(
                        dwp_o[m * P:(m + 1) * P, h * SW:(h + 1) * SW], st[:]
                    )
            nc.sync.dma_start(
                rs_cm[:],
                rs_out[0, 0:S].rearrange("(m p) -> p m", p=P),
            )
            nc.gpsimd.dma_start(loss_o[:], rs_out[0:1, S:S + 1])
            nc.vector.tensor_tensor(btmp_sb[:], rs_cm[:], spr_sb[:], op=ALU.mult)
            nc.vector.tensor_scalar(bcm_sb[:], btmp_sb[:], bf_f, None, op0=ALU.mult)

            # replicate inp / prev_inp across partitions for the dW outers
            inp_row = rowsp.tile([1, I], F32, tag="row", name="inp_row")
            nc.sync.dma_start(inp_row[:], inppi_d[0:1, :])
            inprep = repsp.tile([P, I], F32, tag="rep", name="inprep")
            nc.gpsimd.partition_broadcast(inprep[:], inp_row[:])
            pinp_row = rowsp.tile([1, I], F32, tag="row", name="pinp_row")
            nc.sync.dma_start(pinp_row[:], inppi_d[1:2, :])
            pinprep = repsp.tile([P, I], F32, tag="rep", name="pinprep")
            nc.gpsimd.partition_broadcast(pinprep[:], pinp_row[:])

            # ---- phase 5: dW rows = a*inp + b*prev_inp  (ACT + DVE)
            for m in range(MC):
                for h in range(NH):
                    st = stagep.tile([P, SW], F32, tag="stage",
                                     name=f"std{m}_{h}")
                    nc.scalar.activation(
                        st[:], inprep[:, h * SW:(h + 1) * SW], ACTF.Copy,
                        scale=acm_sb[:, m:m + 1],
                    )
                    nc.vector.scalar_tensor_tensor(
                        st[:], pinprep[:, h * SW:(h + 1) * SW],
                        bcm_sb[:, m:m + 1], st[:],
                        op0=ALU.mult, op1=ALU.add,
                    )
                    nc.gpsimd.dma_start(
                        dw_o[m * P:(m + 1) * P, h * SW:(h + 1) * SW], st[:]
                    )

            # ---- phase 6: remaining small output
            nc.gpsimd.dma_start(fbn_o[:], fbn_sb[:])

    nc.compile()
    return nc


_CACHE = {}


def _get_module(H, I, bf):
    key = (H, I, float(bf))
    if key not in _CACHE:
        _CACHE[key] = build(H=H, I=I, bf=float(bf))
    return _CACHE[key]


def _cm(vec, S):
    """length-S vector -> [128, S/128] column-major tile."""
    return np.ascontiguousarray(vec.reshape(S // P, P).T)


def make_in_maps(inputs, H, I):
    """Host-side sharding: build the 8 per-core input dicts."""
    S = H // NCORES
    f32 = lambda x: np.ascontiguousarray(np.asarray(x, dtype=np.float32))
    inp = f32(inputs["inp"])
    mem = f32(inputs["mem"])
    prev_mem = f32(inputs["prev_mem"])
    prev_inp = f32(inputs["prev_inp"])
    prev_spk = f32(inputs["prev_spk"])
    feedback = f32(inputs["feedback"])
    W_fc = np.asarray(inputs["W_fc"], dtype=np.float32)
    W_pred = np.asarray(inputs["W_pred"], dtype=np.float32)

    inp_cm = _cm(inp, I)
    inp_pi = np.ascontiguousarray(np.stack([inp, prev_inp]))
    pspk = prev_spk[None]

    in_maps = []
    for s in range(NCORES):
        sl = slice(s * S, (s + 1) * S)
        in_maps.append({
            "wfcT": np.ascontiguousarray(W_fc[sl, :].T),
            "wpred": np.ascontiguousarray(W_pred[sl, :]),
            "inp_cm": inp_cm,
            "inp_pi": inp_pi,
            "mem_cm": _cm(mem[sl], S),
            "pmem_cm": _cm(prev_mem[sl], S),
            "fbk_cm": _cm(feedback[sl], S),
            "pspk": pspk,
        })
    return in_maps


def assemble(results, H, I):
    """Gather per-core outputs back into full-shape reference outputs."""
    uncm = lambda t: np.asarray(t).T.reshape(-1)
    spk = np.concatenate([uncm(results[s]["spk_o"]) for s in range(NCORES)])
    mem_out = np.concatenate([uncm(results[s]["memout_o"]) for s in range(NCORES)])
    loss = np.asarray(results[0]["loss_o"][0, 0], dtype=np.float32)
    dW = np.concatenate([results[s]["dw_o"] for s in range(NCORES)], axis=0)
    dW_pred = np.concatenate([results[s]["dwp_o"] for s in range(NCORES)], axis=0)
    fb_new = np.concatenate([uncm(results[s]["fbn_o"]) for s in range(NCORES)])
    return spk, mem_out, loss, dW, dW_pred, fb_new


def run(inputs, **spmd_kwargs):
    W_fc = np.asarray(inputs["W_fc"])
    H, I = W_fc.shape
    bf = float(np.asarray(inputs["bf"]))
    nc = _get_module(H, I, bf)
    in_maps = make_in_maps(inputs, H, I)
    res = run_bass_kernel_spmd(
        nc, in_maps, core_ids=list(range(NCORES)), **spmd_kwargs
    )
    return assemble(res.results, H, I), res


def kernel(**inputs):
    outs, _ = run(inputs)
    return outs
